# revision 28
# baseline (speedup 1.0000x reference)
"""Trainium2 Bass kernel for CustomFlashAttention (B=2, S=2048, D=2048, H=16).

Sharding over 8 NeuronCores: core c handles batch b=c//4 and head-group
hg=c%4 (4 heads of 128 dims = feature cols [hg*512,(hg+1)*512)).
Per core: QKV projections for its cols, causal flash attention for its 4
heads, partial output projection; host sums the 4 bf16 partials per batch.

All matmuls run in bf16 (same 1 col/cycle PE rate as fp32r, but halves
DMA, lets x stay resident for a single projection pass, and lifts the
fp32r >=256-free-dim restriction on diagonal tiles). fp32 accumulation
in PSUM throughout; measured end-to-end max-rel-error ~4e-3.

Softmax denominators come from "ones" matmuls over the exp(S^T) tiles,
col-tiled 4x (32-wide strips at tile_position (0,32j), value 1/32) so 4
key-tiles reduce concurrently; a GPSIMD partition_all_reduce collapses
the per-strip partials into replicated row sums. Attention output is
evicted UNNORMALIZED (frees its PSUM bank immediately) and scaled by
1/rowsum in SBUF once the reduction chain completes.

Emission is software-pipelined: scores tiles are woven between
projection / output-projection quanta so the ACT engine's exp
(~650ns/tile, the latent bottleneck) never stalls the in-order PE
queue; PV tiles trail their scores tiles by a fixed lag; projections
inside the weave run as half-sweeps (2 heads x all kt) so only two PSUM
banks at a time are held, leaving banks for scores/PV/rowsum/outproj.
K(3)'s h2/h3 columns and outproj(2) are deferred into the final step so
qb3's large attention block still has partner work to weave against.

PSUM banks: B0/B1 + B2/B3 proj half-sweeps & outproj (sequential reuse),
B4/B5 scores rotation, B6 PV accumulator, B7 rowsum strips.
"""

import os
import numpy as np
import ml_dtypes

import concourse.bacc as bacc
import concourse.mybir as mybir
import concourse.tile as tile
from concourse.bass_isa import ReduceOp
from concourse.bass_utils import run_bass_kernel_spmd

B = 2
S = 2048
D = 2048
H_PER_CORE = 4
DC = 512          # feature cols per core (4 heads * 128)
HD = 128          # head dim
P = 128
TB = 512          # token block
N_TB = S // TB    # 4
N_KT = D // P     # 16 contraction tiles
BF16 = mybir.dt.bfloat16
FP32 = mybir.dt.float32
NEG = -30000.0

LAG = 3           # PV tile lag behind its scores tile (in S positions)
USE_STRIPS = True # col-tiled ones matmuls (else serial full-array)

LAST_RESULTS = None  # BassKernelResults from the most recent run (for test.py)


def build_bass(causal: bool):
    nc = bacc.Bacc(None, target_bir_lowering=False, debug=False)

    xT_d = nc.dram_tensor("xT", [D, S], BF16, kind="ExternalInput")
    wqT_d = nc.dram_tensor("wqT", [D, DC], BF16, kind="ExternalInput")
    wkT_d = nc.dram_tensor("wkT", [D, DC], BF16, kind="ExternalInput")
    wvT_d = nc.dram_tensor("wvT", [D, DC], BF16, kind="ExternalInput")
    woT_d = nc.dram_tensor("woT", [DC, D], BF16, kind="ExternalInput")
    gm_d = nc.dram_tensor("gmask", [P, P], FP32, kind="ExternalInput")
    out_d = nc.dram_tensor("out", [S, D], BF16, kind="ExternalOutput")

    x_r = xT_d.rearrange("(ko p) t -> p ko t", p=P)     # [128, 16, 2048]
    wq_r = wqT_d.rearrange("(ko p) m -> p ko m", p=P)   # [128, 16, 512]
    wk_r = wkT_d.rearrange("(ko p) m -> p ko m", p=P)
    wv_r = wvT_d.rearrange("(ko p) m -> p ko m", p=P)
    wo_r = woT_d.rearrange("(h p) n -> p h n", p=P)     # [128, 4, 2048]

    Exp = mybir.ActivationFunctionType.Exp
    ADD = mybir.AluOpType.add
    MULT = mybir.AluOpType.mult

    with tile.TileContext(nc) as tc:
        with tc.tile_pool(name="persist", bufs=1) as persist, \
             tc.tile_pool(name="xt", bufs=2) as xtp, \
             tc.tile_pool(name="pt", bufs=26) as ptp, \
             tc.tile_pool(name="zz", bufs=2) as zzp, \
             tc.tile_pool(name="ot", bufs=13) as otp, \
             tc.tile_pool(name="ob", bufs=2) as obp, \
             tc.tile_pool(name="ps", bufs=1, space="PSUM") as psp:

            qt_s = persist.tile([P, H_PER_CORE, S], BF16, tag="qt")
            kt_s = persist.tile([P, H_PER_CORE, S], BF16, tag="kt")
            v_s = persist.tile([P, N_KT, DC], BF16, tag="v")
            wq_s = persist.tile([P, N_KT, DC], BF16, tag="wq")
            wk_s = persist.tile([P, N_KT, DC], BF16, tag="wk")
            wv_s = persist.tile([P, N_KT, DC], BF16, tag="wv")
            wo_s = persist.tile([P, H_PER_CORE, D], BF16, tag="wo")
            gm_s = persist.tile([P, P], FP32, tag="gm")
            on32 = persist.tile([P, 32], BF16, tag="on32")
            on1 = persist.tile([P, P], BF16, tag="on1")
            on1r = persist.tile([P, P], mybir.dt.float32r, tag="on1r")
            on1f = persist.tile([P, P], FP32, tag="on1f")

            nc.any.memset(on32[:], 1.0 / 32)
            nc.any.memset(on1[:], 1.0)
            nc.any.memset(on1f[:], 1.0)
            nc.vector.tensor_copy(out=on1r[:], in_=on1f[:])

            def bank(tag, name):
                return psp.tile([P, TB], FP32, tag=tag, name=name)

            gcur = [0]     # global weave S-index
            gpending = []  # (due_S_idx, closure): ops deferred so the
                           # GPSIMD rowsum reduce never head-of-line
                           # blocks the DVE queue via reciprocal

            def flush_pending(limit=None):
                while gpending and (limit is None
                                    or gpending[0][0] <= limit):
                    gpending.pop(0)[1]()

            # ---- DMA emitters -------------------------------------------
            xts = [None] * N_TB

            def xt_dma(i):
                t = xtp.tile([P, N_KT, TB], BF16, tag="xt", name=f"xt{i}")
                for c in range(4):
                    nc.sync.dma_start(
                        t[:, 4 * c:4 * c + 4, :],
                        x_r[:, 4 * c:4 * c + 4, i * TB:(i + 1) * TB])
                xts[i] = t

            # ---- projection sweeps --------------------------------------
            # kind in {q,k,v}; js = output indices (heads for q/k, token
            # tiles for v); tags = psum bank tags (one per j). Returns one
            # quantum (closure) per kt; evicts attached to the last one.
            def sweep_quanta(kind, i, js, tags):
                w_s = {"q": wq_s, "k": wk_s, "v": wv_s}[kind]
                banks_ = {}

                def quantum(kt):
                    def emit():
                        xt = xts[i]
                        for idx, j in enumerate(js):
                            if kt == 0:
                                banks_[j] = bank(tags[idx],
                                                 f"{kind}{i}_{j}")
                            ps_ = banks_[j]
                            if kind == "v":
                                nc.tensor.matmul(
                                    ps_[:],
                                    xt[:, kt, j * P:(j + 1) * P],
                                    w_s[:, kt, :],
                                    start=(kt == 0), stop=(kt == N_KT - 1))
                            else:
                                nc.tensor.matmul(
                                    ps_[:],
                                    w_s[:, kt, j * HD:(j + 1) * HD],
                                    xt[:, kt, :],
                                    start=(kt == 0), stop=(kt == N_KT - 1))
                        if kt == N_KT - 1:
                            for j in js:
                                eng = nc.vector
                                if kind == "q":
                                    eng.tensor_copy(
                                        out=qt_s[:, j, i * TB:(i + 1) * TB],
                                        in_=banks_[j][:])
                                elif kind == "k":
                                    eng.tensor_copy(
                                        out=kt_s[:, j, i * TB:(i + 1) * TB],
                                        in_=banks_[j][:])
                                else:
                                    eng.tensor_copy(
                                        out=v_s[:, i * 4 + j, :],
                                        in_=banks_[j][:])
                    return emit
                return [quantum(kt) for kt in range(N_KT)]

            def full_sweep(kind, i):
                # prologue only: all 4 outputs at once on B0-B3
                for qn in sweep_quanta(kind, i, [0, 1, 2, 3],
                                       ["B0", "B1", "B2", "B3"]):
                    qn()

            def half_sweeps(kind, i):
                qa = sweep_quanta(kind, i, [0, 1], ["B0", "B1"])
                qb = sweep_quanta(kind, i, [2, 3], ["B2", "B3"])
                return qa + qb

            # ---- output projection (per qb) -----------------------------
            ots_all = [[None] * H_PER_CORE for _ in range(N_TB)]

            def outproj_quanta(i):
                obs = {}

                def quantum(g):
                    tt, nb = divmod(g, 4)

                    def emit():
                        if nb == 0:
                            obs[tt] = obp.tile([P, 4, TB], BF16, tag="ob",
                                               name=f"ob{i}_{tt}")
                        px = bank("B2" if g % 2 == 0 else "B3",
                                  f"x{i}_{tt}_{nb}")
                        for h in range(H_PER_CORE):
                            nc.tensor.matmul(
                                px[:],
                                ots_all[i][h][:, tt * P:(tt + 1) * P],
                                wo_s[:, h, nb * TB:(nb + 1) * TB],
                                start=(h == 0), stop=(h == H_PER_CORE - 1))
                        nc.vector.tensor_copy(out=obs[tt][:, nb, :],
                                              in_=px[:])
                        if nb == 3:
                            row0 = i * TB + tt * P
                            nc.sync.dma_start(out_d[row0:row0 + P, :],
                                              obs[tt][:])
                    return emit
                return [quantum(g) for g in range(16)]

            # ---- attention tiles (per qb) -------------------------------
            def attn_items(i):
                nkt = 4 * i + 4 if causal else N_KT
                pts = {}
                pso = {}
                psn = {}
                sidx = [0]

                def s0_of(kt):
                    if causal and kt >= 4 * i:
                        return (kt - 4 * i) * P
                    return 0

                def make_S(h, kt):
                    def emit():
                        s0 = s0_of(kt)
                        rot = ("B4", "B5")[sidx[0] % 2]
                        sidx[0] += 1
                        ps_s = bank(rot, f"s{i}_{h}_{kt}")
                        nc.tensor.matmul(
                            ps_s[:, s0:],
                            kt_s[:, h, kt * P:(kt + 1) * P],
                            qt_s[:, h, i * TB + s0:(i + 1) * TB],
                            start=True, stop=True)
                        ptile = ptp.tile([P, TB], BF16, tag="p",
                                         name=f"p{i}_{h}_{kt}")
                        if causal and kt >= 4 * i:
                            nc.vector.tensor_tensor(
                                ps_s[:, s0:s0 + P], ps_s[:, s0:s0 + P],
                                gm_s[:], ADD)
                        nc.scalar.activation(ptile[:, s0:], ps_s[:, s0:], Exp)
                        pts[(h, kt)] = ptile
                    return emit

                def make_P(h, kt):
                    # serial rowsum (full-array ones, psn ends up fully
                    # replicated -> reciprocal straight from PSUM): used
                    # for qb0 (whose strips would leave unwritten psum
                    # columns) and when strips are disabled. Must stay
                    # out of the strips steps: its kt0 psn allocation
                    # would invert the B7 WAR order against a deferred
                    # replicate-matmul and deadlock the schedule.
                    serial = (causal and i == 0) or not USE_STRIPS

                    def emit():
                        s0 = s0_of(kt)
                        if kt == 0:
                            pso[h] = bank("B6", f"o{i}_{h}")
                            if serial:
                                psn[h] = bank("B7", f"n{i}_{h}")
                        nc.tensor.matmul(
                            pso[h][:, s0:],
                            v_s[:, kt, h * HD:(h + 1) * HD],
                            pts[(h, kt)][:, s0:],
                            start=(kt == 0), stop=(kt == nkt - 1))
                        if serial:
                            nc.tensor.matmul(
                                psn[h][:, s0:], on1[:],
                                pts[(h, kt)][:, s0:],
                                start=(kt == 0), stop=(kt == nkt - 1))
                        elif kt % 4 == 3:
                            G = nkt // 4
                            g = kt // 4
                            if g == 0:
                                psn[h] = bank("B7", f"n{i}_{h}")
                            for j in range(4):
                                kj = 4 * g + j
                                sj = s0_of(kj)
                                nc.tensor.matmul(
                                    psn[h][32 * j:32 * j + 32, sj:],
                                    on32[:],
                                    pts[(h, kj)][:, sj:],
                                    start=(g == 0), stop=(g == G - 1),
                                    tile_position=(0, 32 * j))
                        if kt == nkt - 1:
                            ot_t = otp.tile([P, TB], BF16, tag="ot",
                                            name=f"ot{i}_{h}")
                            rc = zzp.tile([P, TB], FP32, tag="rc",
                                          name=f"rc{i}_{h}")
                            if serial:
                                nc.vector.reciprocal_approx_fast(
                                    out=rc[:], in_=psn[h][:])
                                nc.vector.tensor_tensor(ot_t[:], pso[h][:],
                                                        rc[:], MULT)
                            else:
                                # evict PV unnormalized (frees B6 fast).
                                # The per-strip partial rowsums collapse
                                # via a second small PE matmul (ones/32 x
                                # z, replicated output) instead of a
                                # GPSIMD reduce: deferred a few weave
                                # slots so nothing head-of-line blocks.
                                nc.vector.tensor_copy(out=ot_t[:],
                                                      in_=pso[h][:])
                                z = zzp.tile([P, TB], mybir.dt.float32r,
                                             tag="z", name=f"z{i}_{h}")
                                nc.vector.tensor_copy(out=z[:],
                                                      in_=psn[h][:])

                                def rep(z=z, rc=rc, hh=h):
                                    pr = bank("B7", f"r{i}_{hh}")
                                    nc.tensor.matmul(pr[:], on1r[:], z[:],
                                                     start=True, stop=True)
                                    nc.vector.reciprocal_approx_fast(
                                        out=rc[:], in_=pr[:])

                                def finish(rc=rc, ot_t=ot_t):
                                    nc.vector.tensor_tensor(
                                        ot_t[:], ot_t[:], rc[:], MULT)
                                gpending.append((gcur[0] + 2, rep))
                                gpending.append((gcur[0] + 4, finish))
                            ots_all[i][h] = ot_t
                    return emit

                S_items = [make_S(h, kt)
                           for h in range(H_PER_CORE) for kt in range(nkt)]
                P_items = [make_P(h, kt)
                           for h in range(H_PER_CORE) for kt in range(nkt)]
                return S_items, P_items

            # ---- weave one step -----------------------------------------
            def weave(S_items, P_items, partners):
                f = len(partners) / len(S_items)
                credit = 0.0
                pi = 0
                for j in range(len(S_items)):
                    gcur[0] += 1
                    S_items[j]()
                    if j >= LAG:
                        P_items[pi]()
                        pi += 1
                    flush_pending(gcur[0])
                    credit += f
                    while credit >= 1.0 and partners:
                        partners.pop(0)()
                        credit -= 1.0
                while pi < len(P_items):
                    P_items[pi]()
                    pi += 1
                while partners:
                    partners.pop(0)()

            def step(i):
                # partner order keeps each bank pair's users sequential:
                # qa(B0/B1), outproj(B2/B3), qb(B2/B3), then v/k reuse.
                # K(i+1)'s h2/h3 columns are deferred into the LAST step
                # (attn(3)'s diag tiles for h2/h3 come late enough) so
                # qb3 still has partner work to weave against.
                S_items, P_items = attn_items(i)
                partners = []
                if i < N_TB - 1:
                    qa = sweep_quanta("q", i + 1, [0, 1], ["B0", "B1"])
                    qb = sweep_quanta("q", i + 1, [2, 3], ["B2", "B3"])
                    partners += qa
                    if i == 1:
                        partners += outproj_quanta(0)
                    partners += qb
                    partners += half_sweeps("v", i + 1)
                    if i + 1 < N_TB - 1:
                        partners += half_sweeps("k", i + 1)
                    else:
                        partners += sweep_quanta("k", i + 1, [0, 1],
                                                 ["B0", "B1"])
                tail_groups = []
                if i == N_TB - 1:
                    partners += sweep_quanta("k", i, [2], ["B0"])
                    partners += sweep_quanta("k", i, [3], ["B1"])
                    partners += outproj_quanta(i - 2)
                    tail_groups = outproj_quanta(i - 1)
                    # hold back the last two groups: emitted after the
                    # PV drain so they cover the final rowsum chain
                    # before the epilogue outproj starts
                    partners += tail_groups[:-2]
                    tail_groups = tail_groups[-2:]
                weave(S_items, P_items, partners)
                for qn in tail_groups:
                    qn()

            # ================= emission =================
            # prologue DMAs: wq + xt0 interleaved first (fine-grained
            # leading chunks so the first matmul starts ASAP after the
            # ~7.4us framework preamble), then the rest
            xts[0] = xtp.tile([P, N_KT, TB], BF16, tag="xt", name="xt0")
            for lo, hi in ((0, 1), (1, 2), (2, 4), (4, 8), (8, 12),
                           (12, 16)):
                nc.sync.dma_start(wq_s[:, lo:hi, :], wq_r[:, lo:hi, :])
                nc.sync.dma_start(xts[0][:, lo:hi, :],
                                  x_r[:, lo:hi, 0:TB])
            for c in range(4):
                nc.sync.dma_start(wv_s[:, 4 * c:4 * c + 4, :],
                                  wv_r[:, 4 * c:4 * c + 4, :])
            for c in range(4):
                nc.sync.dma_start(wk_s[:, 4 * c:4 * c + 4, :],
                                  wk_r[:, 4 * c:4 * c + 4, :])
            xt_dma(1)
            nc.sync.dma_start(wo_s[:], wo_r[:])
            nc.sync.dma_start(gm_s[:], gm_d[:])

            if causal:
                # prologue projections for tb0 (bare, full sweeps)
                full_sweep("q", 0)
                full_sweep("v", 0)
                full_sweep("k", 0)
                for i in range(N_TB):
                    if i + 2 < N_TB:
                        xt_dma(i + 2)
                    step(i)
                flush_pending()
                for qn in outproj_quanta(N_TB - 1):
                    qn()
            else:
                # non-causal: all projections first, then attention
                for i in range(N_TB):
                    if i >= 2:
                        xt_dma(i)
                    full_sweep("q", i)
                    full_sweep("v", i)
                    full_sweep("k", i)
                for i in range(N_TB):
                    S_items, P_items = attn_items(i)
                    partners = outproj_quanta(i - 1) if i > 0 else []
                    weave(S_items, P_items, partners)
                flush_pending()
                for qn in outproj_quanta(N_TB - 1):
                    qn()

    nc.compile()
    return nc


_BASS_CACHE = {}


def kernel(x, w_q, w_k, w_v, w_o, causal):
    global LAST_RESULTS
    x = np.asarray(x, dtype=np.float32)
    w_q = np.asarray(w_q, dtype=np.float32)
    w_k = np.asarray(w_k, dtype=np.float32)
    w_v = np.asarray(w_v, dtype=np.float32)
    w_o = np.asarray(w_o, dtype=np.float32)
    is_causal = bool(int(causal))

    if is_causal not in _BASS_CACHE:
        _BASS_CACHE[is_causal] = build_bass(is_causal)
    nc = _BASS_CACHE[is_causal]

    bf16 = ml_dtypes.bfloat16
    scale = np.float32(1.0 / np.sqrt(HD))
    gm = np.zeros((P, P), dtype=np.float32)
    ii = np.arange(P)[:, None]
    jj = np.arange(P)[None, :]
    gm[jj < ii] = NEG

    xT = [np.ascontiguousarray(x[b].T).astype(bf16) for b in range(B)]
    in_maps = []
    for c in range(8):
        b, hg = divmod(c, 4)
        cols = slice(hg * DC, (hg + 1) * DC)
        in_maps.append({
            "xT": xT[b],
            "wqT": np.ascontiguousarray(w_q[cols, :].T * scale).astype(bf16),
            "wkT": np.ascontiguousarray(w_k[cols, :].T).astype(bf16),
            "wvT": np.ascontiguousarray(w_v[cols, :].T).astype(bf16),
            "woT": np.ascontiguousarray(w_o[:, cols].T).astype(bf16),
            "gmask": gm,
        })

    trace = bool(os.environ.get("KERNEL_TRACE"))
    try:
        res = run_bass_kernel_spmd(nc, in_maps, list(range(8)), trace=trace)
    except Exception:
        if not trace:
            raise
        res = run_bass_kernel_spmd(nc, in_maps, list(range(8)), trace=False)
    LAST_RESULTS = res

    out = np.zeros((B, S, D), dtype=np.float32)
    for c in range(8):
        b = c // 4
        out[b] += np.asarray(res.results[c]["out"], dtype=np.float32)
    return out


# revision 31
# speedup vs baseline: 1.0002x; 1.0002x over previous
"""Trainium2 Bass kernel for CustomFlashAttention (B=2, S=2048, D=2048, H=16).

Sharding over 8 NeuronCores: core c handles batch b=c//4 and head-group
hg=c%4 (4 heads of 128 dims = feature cols [hg*512,(hg+1)*512)).
Per core: QKV projections for its cols, causal flash attention for its 4
heads, partial output projection; host sums the 4 bf16 partials per batch.

All matmuls run in bf16 (same 1 col/cycle PE rate as fp32r, but halves
DMA, lets x stay resident for a single projection pass, and lifts the
fp32r >=256-free-dim restriction on diagonal tiles). fp32 accumulation
in PSUM throughout; measured end-to-end max-rel-error ~4e-3.

Softmax denominators come from "ones" matmuls over the exp(S^T) tiles,
col-tiled 4x (32-wide strips at tile_position (0,32j), value 1/32) so 4
key-tiles reduce concurrently; a GPSIMD partition_all_reduce collapses
the per-strip partials into replicated row sums. Attention output is
evicted UNNORMALIZED (frees its PSUM bank immediately) and scaled by
1/rowsum in SBUF once the reduction chain completes.

Emission is software-pipelined: scores tiles are woven between
projection / output-projection quanta so the ACT engine's exp
(~650ns/tile, the latent bottleneck) never stalls the in-order PE
queue; PV tiles trail their scores tiles by a fixed lag; projections
inside the weave run as half-sweeps (2 heads x all kt) so only two PSUM
banks at a time are held, leaving banks for scores/PV/rowsum/outproj.
K(3)'s h2/h3 columns and outproj(2) are deferred into the final step so
qb3's large attention block still has partner work to weave against.

PSUM banks: B0/B1 + B2/B3 proj half-sweeps & outproj (sequential reuse),
B4/B5 scores rotation, B6 PV accumulator, B7 rowsum strips.
"""

import os
import numpy as np
import ml_dtypes

import concourse.bacc as bacc
import concourse.mybir as mybir
import concourse.tile as tile
from concourse.bass_isa import ReduceOp
from concourse.bass_utils import run_bass_kernel_spmd

B = 2
S = 2048
D = 2048
H_PER_CORE = 4
DC = 512          # feature cols per core (4 heads * 128)
HD = 128          # head dim
P = 128
TB = 512          # token block
N_TB = S // TB    # 4
N_KT = D // P     # 16 contraction tiles
BF16 = mybir.dt.bfloat16
FP32 = mybir.dt.float32
NEG = -30000.0

LAG = 3           # PV tile lag behind its scores tile (in S positions)
USE_STRIPS = True # col-tiled ones matmuls (else serial full-array)

LAST_RESULTS = None  # BassKernelResults from the most recent run (for test.py)


def build_bass(causal: bool):
    nc = bacc.Bacc(None, target_bir_lowering=False, debug=False)

    xT_d = nc.dram_tensor("xT", [D, S], BF16, kind="ExternalInput")
    wqT_d = nc.dram_tensor("wqT", [D, DC], BF16, kind="ExternalInput")
    wkT_d = nc.dram_tensor("wkT", [D, DC], BF16, kind="ExternalInput")
    wvT_d = nc.dram_tensor("wvT", [D, DC], BF16, kind="ExternalInput")
    woT_d = nc.dram_tensor("woT", [DC, D], BF16, kind="ExternalInput")
    gm_d = nc.dram_tensor("gmask", [P, P], FP32, kind="ExternalInput")
    out_d = nc.dram_tensor("out", [S, D], BF16, kind="ExternalOutput")

    x_r = xT_d.rearrange("(ko p) t -> p ko t", p=P)     # [128, 16, 2048]
    wq_r = wqT_d.rearrange("(ko p) m -> p ko m", p=P)   # [128, 16, 512]
    wk_r = wkT_d.rearrange("(ko p) m -> p ko m", p=P)
    wv_r = wvT_d.rearrange("(ko p) m -> p ko m", p=P)
    wo_r = woT_d.rearrange("(h p) n -> p h n", p=P)     # [128, 4, 2048]

    Exp = mybir.ActivationFunctionType.Exp
    ADD = mybir.AluOpType.add
    MULT = mybir.AluOpType.mult

    with tile.TileContext(nc) as tc:
        with tc.tile_pool(name="persist", bufs=1) as persist, \
             tc.tile_pool(name="xt", bufs=2) as xtp, \
             tc.tile_pool(name="pt", bufs=26) as ptp, \
             tc.tile_pool(name="zz", bufs=2) as zzp, \
             tc.tile_pool(name="ot", bufs=13) as otp, \
             tc.tile_pool(name="ob", bufs=2) as obp, \
             tc.tile_pool(name="ps", bufs=1, space="PSUM") as psp:

            qt_s = persist.tile([P, H_PER_CORE, S], BF16, tag="qt")
            kt_s = persist.tile([P, H_PER_CORE, S], BF16, tag="kt")
            v_s = persist.tile([P, N_KT, DC], BF16, tag="v")
            wq_s = persist.tile([P, N_KT, DC], BF16, tag="wq")
            wk_s = persist.tile([P, N_KT, DC], BF16, tag="wk")
            wv_s = persist.tile([P, N_KT, DC], BF16, tag="wv")
            wo_s = persist.tile([P, H_PER_CORE, D], BF16, tag="wo")
            gm_s = persist.tile([P, P], FP32, tag="gm")
            on32 = persist.tile([P, 32], BF16, tag="on32")
            on1 = persist.tile([P, P], BF16, tag="on1")
            on1r = persist.tile([P, P], mybir.dt.float32r, tag="on1r")
            on1f = persist.tile([P, P], FP32, tag="on1f")

            nc.any.memset(on32[:], 1.0 / 32)
            nc.any.memset(on1[:], 1.0)
            nc.any.memset(on1f[:], 1.0)
            nc.vector.tensor_copy(out=on1r[:], in_=on1f[:])

            def bank(tag, name):
                return psp.tile([P, TB], FP32, tag=tag, name=name)

            gcur = [0]     # global weave S-index
            gpending = []  # (due_S_idx, closure): ops deferred so the
                           # GPSIMD rowsum reduce never head-of-line
                           # blocks the DVE queue via reciprocal

            def flush_pending(limit=None):
                while gpending and (limit is None
                                    or gpending[0][0] <= limit):
                    gpending.pop(0)[1]()

            # ---- DMA emitters -------------------------------------------
            xts = [None] * N_TB

            def xt_dma(i):
                t = xtp.tile([P, N_KT, TB], BF16, tag="xt", name=f"xt{i}")
                for c in range(4):
                    nc.sync.dma_start(
                        t[:, 4 * c:4 * c + 4, :],
                        x_r[:, 4 * c:4 * c + 4, i * TB:(i + 1) * TB])
                xts[i] = t

            # ---- projection sweeps --------------------------------------
            # kind in {q,k,v}; js = output indices (heads for q/k, token
            # tiles for v); tags = psum bank tags (one per j). Returns one
            # quantum (closure) per kt; evicts attached to the last one.
            def sweep_quanta(kind, i, js, tags):
                w_s = {"q": wq_s, "k": wk_s, "v": wv_s}[kind]
                banks_ = {}

                def quantum(kt):
                    def emit():
                        xt = xts[i]
                        for idx, j in enumerate(js):
                            if kt == 0:
                                banks_[j] = bank(tags[idx],
                                                 f"{kind}{i}_{j}")
                            ps_ = banks_[j]
                            if kind == "v":
                                nc.tensor.matmul(
                                    ps_[:],
                                    xt[:, kt, j * P:(j + 1) * P],
                                    w_s[:, kt, :],
                                    start=(kt == 0), stop=(kt == N_KT - 1))
                            else:
                                nc.tensor.matmul(
                                    ps_[:],
                                    w_s[:, kt, j * HD:(j + 1) * HD],
                                    xt[:, kt, :],
                                    start=(kt == 0), stop=(kt == N_KT - 1))
                        if kt == N_KT - 1:
                            for j in js:
                                eng = nc.vector
                                if kind == "q":
                                    eng.tensor_copy(
                                        out=qt_s[:, j, i * TB:(i + 1) * TB],
                                        in_=banks_[j][:])
                                elif kind == "k":
                                    eng.tensor_copy(
                                        out=kt_s[:, j, i * TB:(i + 1) * TB],
                                        in_=banks_[j][:])
                                else:
                                    eng.tensor_copy(
                                        out=v_s[:, i * 4 + j, :],
                                        in_=banks_[j][:])
                    return emit
                return [quantum(kt) for kt in range(N_KT)]

            def full_sweep(kind, i):
                # prologue only: all 4 outputs at once on B0-B3
                for qn in sweep_quanta(kind, i, [0, 1, 2, 3],
                                       ["B0", "B1", "B2", "B3"]):
                    qn()

            def half_sweeps(kind, i):
                qa = sweep_quanta(kind, i, [0, 1], ["B0", "B1"])
                qb = sweep_quanta(kind, i, [2, 3], ["B2", "B3"])
                return qa + qb

            # ---- output projection (per qb) -----------------------------
            ots_all = [[None] * H_PER_CORE for _ in range(N_TB)]

            def outproj_quanta(i):
                obs = {}

                def quantum(g):
                    tt, nb = divmod(g, 4)

                    def emit():
                        if nb == 0:
                            obs[tt] = obp.tile([P, 4, TB], BF16, tag="ob",
                                               name=f"ob{i}_{tt}")
                        px = bank("B2" if g % 2 == 0 else "B3",
                                  f"x{i}_{tt}_{nb}")
                        for h in range(H_PER_CORE):
                            nc.tensor.matmul(
                                px[:],
                                ots_all[i][h][:, tt * P:(tt + 1) * P],
                                wo_s[:, h, nb * TB:(nb + 1) * TB],
                                start=(h == 0), stop=(h == H_PER_CORE - 1))
                        nc.vector.tensor_copy(out=obs[tt][:, nb, :],
                                              in_=px[:])
                        if nb == 3:
                            row0 = i * TB + tt * P
                            nc.sync.dma_start(out_d[row0:row0 + P, :],
                                              obs[tt][:])
                    return emit
                return [quantum(g) for g in range(16)]

            # ---- attention tiles (per qb) -------------------------------
            def attn_items(i):
                nkt = 4 * i + 4 if causal else N_KT
                pts = {}
                pso = {}
                psn = {}
                sidx = [0]

                def s0_of(kt):
                    if causal and kt >= 4 * i:
                        return (kt - 4 * i) * P
                    return 0

                def make_S(h, kt):
                    def emit():
                        s0 = s0_of(kt)
                        rot = ("B4", "B5")[sidx[0] % 2]
                        sidx[0] += 1
                        ps_s = bank(rot, f"s{i}_{h}_{kt}")
                        nc.tensor.matmul(
                            ps_s[:, s0:],
                            kt_s[:, h, kt * P:(kt + 1) * P],
                            qt_s[:, h, i * TB + s0:(i + 1) * TB],
                            start=True, stop=True)
                        ptile = ptp.tile([P, TB], BF16, tag="p",
                                         name=f"p{i}_{h}_{kt}")
                        if causal and kt >= 4 * i:
                            nc.vector.tensor_tensor(
                                ps_s[:, s0:s0 + P], ps_s[:, s0:s0 + P],
                                gm_s[:], ADD)
                        nc.scalar.activation(ptile[:, s0:], ps_s[:, s0:], Exp)
                        pts[(h, kt)] = ptile
                    return emit

                def make_P(h, kt):
                    # serial rowsum (full-array ones, psn ends up fully
                    # replicated -> reciprocal straight from PSUM): used
                    # for qb0 (whose strips would leave unwritten psum
                    # columns) and when strips are disabled. Must stay
                    # out of the strips steps: its kt0 psn allocation
                    # would invert the B7 WAR order against a deferred
                    # replicate-matmul and deadlock the schedule.
                    serial = (causal and i == 0) or not USE_STRIPS

                    def emit():
                        s0 = s0_of(kt)
                        if kt == 0:
                            pso[h] = bank("B6", f"o{i}_{h}")
                            if serial:
                                psn[h] = bank("B7", f"n{i}_{h}")
                        nc.tensor.matmul(
                            pso[h][:, s0:],
                            v_s[:, kt, h * HD:(h + 1) * HD],
                            pts[(h, kt)][:, s0:],
                            start=(kt == 0), stop=(kt == nkt - 1))
                        if serial:
                            nc.tensor.matmul(
                                psn[h][:, s0:], on1[:],
                                pts[(h, kt)][:, s0:],
                                start=(kt == 0), stop=(kt == nkt - 1))
                        elif kt % 4 == 3:
                            G = nkt // 4
                            g = kt // 4
                            if g == 0:
                                psn[h] = bank("B7", f"n{i}_{h}")
                            for j in range(4):
                                kj = 4 * g + j
                                sj = s0_of(kj)
                                nc.tensor.matmul(
                                    psn[h][32 * j:32 * j + 32, sj:],
                                    on32[:],
                                    pts[(h, kj)][:, sj:],
                                    start=(g == 0), stop=(g == G - 1),
                                    tile_position=(0, 32 * j))
                        if kt == nkt - 1:
                            ot_t = otp.tile([P, TB], BF16, tag="ot",
                                            name=f"ot{i}_{h}")
                            rc = zzp.tile([P, TB], FP32, tag="rc",
                                          name=f"rc{i}_{h}")
                            if serial:
                                nc.vector.reciprocal_approx_fast(
                                    out=rc[:], in_=psn[h][:])
                                nc.vector.tensor_tensor(ot_t[:], pso[h][:],
                                                        rc[:], MULT)
                            else:
                                # evict PV unnormalized (frees B6 fast).
                                # The per-strip partial rowsums collapse
                                # via a second small PE matmul (ones/32 x
                                # z, replicated output) instead of a
                                # GPSIMD reduce: deferred a few weave
                                # slots so nothing head-of-line blocks.
                                nc.vector.tensor_copy(out=ot_t[:],
                                                      in_=pso[h][:])
                                z = zzp.tile([P, TB], mybir.dt.float32r,
                                             tag="z", name=f"z{i}_{h}")
                                nc.vector.tensor_copy(out=z[:],
                                                      in_=psn[h][:])

                                def rep(z=z, rc=rc, hh=h):
                                    pr = bank("B7", f"r{i}_{hh}")
                                    nc.tensor.matmul(pr[:], on1r[:], z[:],
                                                     start=True, stop=True)
                                    nc.vector.reciprocal_approx_fast(
                                        out=rc[:], in_=pr[:])

                                def finish(rc=rc, ot_t=ot_t):
                                    nc.vector.tensor_tensor(
                                        ot_t[:], ot_t[:], rc[:], MULT)
                                gpending.append((gcur[0] + 2, rep))
                                gpending.append((gcur[0] + 4, finish))
                            ots_all[i][h] = ot_t
                    return emit

                S_items = [make_S(h, kt)
                           for h in range(H_PER_CORE) for kt in range(nkt)]
                P_items = [make_P(h, kt)
                           for h in range(H_PER_CORE) for kt in range(nkt)]
                return S_items, P_items

            # ---- weave one step -----------------------------------------
            def weave(S_items, P_items, partners):
                f = len(partners) / len(S_items)
                credit = 0.0
                pi = 0
                for j in range(len(S_items)):
                    gcur[0] += 1
                    S_items[j]()
                    if j >= LAG:
                        P_items[pi]()
                        pi += 1
                    flush_pending(gcur[0])
                    credit += f
                    while credit >= 1.0 and partners:
                        partners.pop(0)()
                        credit -= 1.0
                while pi < len(P_items):
                    P_items[pi]()
                    pi += 1
                while partners:
                    partners.pop(0)()

            def step(i):
                # partner order keeps each bank pair's users sequential:
                # qa(B0/B1), outproj(B2/B3), qb(B2/B3), then v/k reuse.
                # K(i+1)'s h2/h3 columns are deferred into the LAST step
                # (attn(3)'s diag tiles for h2/h3 come late enough) so
                # qb3 still has partner work to weave against.
                S_items, P_items = attn_items(i)
                partners = []
                if i < N_TB - 1:
                    qa = sweep_quanta("q", i + 1, [0, 1], ["B0", "B1"])
                    qb = sweep_quanta("q", i + 1, [2, 3], ["B2", "B3"])
                    partners += qa
                    if i == 1:
                        partners += outproj_quanta(0)
                    partners += qb
                    partners += half_sweeps("v", i + 1)
                    if i + 1 < N_TB - 1:
                        partners += half_sweeps("k", i + 1)
                    else:
                        partners += sweep_quanta("k", i + 1, [0, 1],
                                                 ["B0", "B1"])
                tail_groups = []
                if i == N_TB - 1:
                    partners += sweep_quanta("k", i, [2], ["B0"])
                    partners += sweep_quanta("k", i, [3], ["B1"])
                    partners += outproj_quanta(i - 2)
                    tail_groups = outproj_quanta(i - 1)
                    # hold back the last two groups: emitted after the
                    # PV drain so they cover the final rowsum chain
                    # before the epilogue outproj starts
                    partners += tail_groups[:-2]
                    tail_groups = tail_groups[-2:]
                weave(S_items, P_items, partners)
                for qn in tail_groups:
                    qn()

            # ================= emission =================
            # prologue DMAs: wq + xt0 interleaved first (fine-grained
            # leading chunks so the first matmul starts ASAP after the
            # ~7.4us framework preamble), then the rest
            xts[0] = xtp.tile([P, N_KT, TB], BF16, tag="xt", name="xt0")
            for lo, hi in ((0, 1), (1, 2), (2, 4), (4, 8), (8, 12),
                           (12, 16)):
                nc.sync.dma_start(wq_s[:, lo:hi, :], wq_r[:, lo:hi, :])
                nc.sync.dma_start(xts[0][:, lo:hi, :],
                                  x_r[:, lo:hi, 0:TB])
            for c in range(4):
                nc.sync.dma_start(wv_s[:, 4 * c:4 * c + 4, :],
                                  wv_r[:, 4 * c:4 * c + 4, :])
            for c in range(4):
                nc.sync.dma_start(wk_s[:, 4 * c:4 * c + 4, :],
                                  wk_r[:, 4 * c:4 * c + 4, :])
            xt_dma(1)
            nc.sync.dma_start(wo_s[:], wo_r[:])
            nc.sync.dma_start(gm_s[:], gm_d[:])

            if causal:
                # prologue projections for tb0 (bare, full sweeps)
                full_sweep("q", 0)
                full_sweep("v", 0)
                full_sweep("k", 0)
                for i in range(N_TB):
                    if i + 2 < N_TB:
                        xt_dma(i + 2)
                    step(i)
                flush_pending()
                for qn in outproj_quanta(N_TB - 1):
                    qn()
            else:
                # non-causal: all projections first, then attention
                for i in range(N_TB):
                    if i >= 2:
                        xt_dma(i)
                    full_sweep("q", i)
                    full_sweep("v", i)
                    full_sweep("k", i)
                for i in range(N_TB):
                    S_items, P_items = attn_items(i)
                    partners = outproj_quanta(i - 1) if i > 0 else []
                    weave(S_items, P_items, partners)
                flush_pending()
                for qn in outproj_quanta(N_TB - 1):
                    qn()

    nc.compile()
    return nc


_BASS_CACHE = {}


def kernel(x, w_q, w_k, w_v, w_o, causal):
    global LAST_RESULTS
    x = np.asarray(x, dtype=np.float32)
    w_q = np.asarray(w_q, dtype=np.float32)
    w_k = np.asarray(w_k, dtype=np.float32)
    w_v = np.asarray(w_v, dtype=np.float32)
    w_o = np.asarray(w_o, dtype=np.float32)
    is_causal = bool(int(causal))

    if is_causal not in _BASS_CACHE:
        _BASS_CACHE[is_causal] = build_bass(is_causal)
    nc = _BASS_CACHE[is_causal]

    bf16 = ml_dtypes.bfloat16
    scale = np.float32(1.0 / np.sqrt(HD))
    gm = np.zeros((P, P), dtype=np.float32)
    ii = np.arange(P)[:, None]
    jj = np.arange(P)[None, :]
    gm[jj < ii] = NEG

    xT = [np.ascontiguousarray(x[b].T).astype(bf16) for b in range(B)]
    in_maps = []
    for c in range(8):
        b, hg = divmod(c, 4)
        cols = slice(hg * DC, (hg + 1) * DC)
        in_maps.append({
            "xT": xT[b],
            "wqT": np.ascontiguousarray(w_q[cols, :].T * scale).astype(bf16),
            "wkT": np.ascontiguousarray(w_k[cols, :].T).astype(bf16),
            "wvT": np.ascontiguousarray(w_v[cols, :].T).astype(bf16),
            "woT": np.ascontiguousarray(w_o[:, cols].T).astype(bf16),
            "gmask": gm,
        })

    trace = bool(os.environ.get("KERNEL_TRACE"))
    try:
        res = run_bass_kernel_spmd(nc, in_maps, list(range(8)), trace=trace)
    except Exception:
        if not trace:
            raise
        res = run_bass_kernel_spmd(nc, in_maps, list(range(8)), trace=False)
    LAST_RESULTS = res

    out = np.zeros((B, S, D), dtype=np.float32)
    for c in range(8):
        b = c // 4
        out[b] += np.asarray(res.results[c]["out"], dtype=np.float32)
    return out
